# revision 5
# baseline (speedup 1.0000x reference)
"""Trainium2 Bass kernel for a 2-layer GCN encoder (AssemblyQueryEncoder).

Reference computation (PyG-style GCNConv x2 + global mean pool + linear + L2norm):
    h1 = relu(gcnconv(x, W1, b1));  h2 = relu(gcnconv(h1, W2, b2))
    g  = segment_mean(h2, batch) @ Wl + bl;  out = g / max(||g||_2, eps)

Distribution over 8 NeuronCores:
  - Nodes sharded contiguously (5120 padded/core); each core owns the incoming
    edges of its nodes (destination partitioning).
  - Per layer: local transform h = x @ W (bf16); the AllGather of h is SPLIT
    into NQ quarter-collectives.  Aggregation runs PASS-MAJOR (one pass per
    source quarter), accumulating per-block partial sums into an SBUF f32
    accumulator, so aggregation of quarter q overlaps the AllGather of later
    quarters.  Pass 0 also carries the self-loop diagonal matmul and the
    rank-1 bias matmul (no table dependency); the last pass ends in Relu.
  - Selection matrices are built ON-CHIP by the vector engine from compact
    per-edge metadata (dst column + edge norm, 2B each per edge): one
    is_equal against a repeated iota plus one multiply per chunk, instead of
    streaming dense bf16 selection matrices from DRAM (halves HBM traffic).
  - Layer-1 aggregation computes the TRANSPOSED output h1T[f, node] directly
    (gathered tile as lhsT, selection matrix as rhs), so layer 2's transform
    consumes it with no PE transpose pass.  Layer-2 transform+AllGather are
    emitted from a callback inside layer-1's last aggregation pass, quarter
    by quarter, hiding the inter-layer collective latency; pooling matmuls
    ride the same callback during layer 2.
  - Pooled per-graph sums (1/count folded into the pooling matrix) are
    AllReduced ([128,64]); final linear + L2 norm computed redundantly in f32.
"""

import sys

sys.path.insert(0, "/opt/trn_rl_repo")

import numpy as np

P = 128  # partitions


def _cdiv(a, b):
    return (a + b - 1) // b


def _cdiv_arr(a, b):
    return (a + b - 1) // b


class GCNConfig:
    def __init__(self, n_nodes=40000, n_graphs=64, d_in=128, d_hid=128, d_out=64,
                 n_cores=8, chunk_tiles=8):
        self.n_nodes = n_nodes
        self.n_graphs = n_graphs
        self.d_in = d_in
        self.d_hid = d_hid
        self.d_out = d_out
        self.n_cores = n_cores
        self.chunk_tiles = chunk_tiles
        self.nloc = _cdiv(n_nodes, n_cores * P) * P  # padded nodes per core
        self.npad = self.nloc * n_cores
        self.nblk = self.nloc // P  # 128-node blocks per core
        # number of AllGather splits; quarter tables must stay int16-addressable
        self.nq = 4 if (self.nblk % 4 == 0
                        and (self.nloc // 4) * n_cores <= 32768) else 1
        assert (self.nloc // self.nq) * n_cores <= 32768


def _wrap_idx(flat):
    """dma_gather index layout: element i -> [i % 16, i // 16], x8 partitions."""
    n = flat.shape[0]
    assert n % 16 == 0
    arr = np.zeros((16, n // 16), np.int16)
    arr[np.arange(n) % 16, np.arange(n) // 16] = flat
    return np.tile(arr, (8, 1))


def preprocess(cfg, x, edge_index, batch):
    """Host-side index preprocessing. Edges are grouped per core by destination
    block and split into nq streams by source quarter; each (block, stream)
    list is padded to a tile multiple shared by all cores. Per-edge metadata
    (dst column within block, edge norm) is packed per tile slot; selection
    matrices are built on-chip."""
    import ml_dtypes
    bfd = ml_dtypes.bfloat16

    n, nc_ = cfg.n_nodes, cfg.n_cores
    nq, qs = cfg.nq, cfg.nloc // cfg.nq
    src_a = np.asarray(edge_index[0], dtype=np.int64)
    dst_a = np.asarray(edge_index[1], dtype=np.int64)
    batch = np.asarray(batch, dtype=np.int64)

    deg = np.bincount(dst_a, minlength=n).astype(np.float64) + 1.0
    dinv = 1.0 / np.sqrt(deg)
    nrm_a = (dinv[src_a] * dinv[dst_a]).astype(np.float32)
    dinv2 = (dinv * dinv).astype(np.float32)

    # source quarter + row within that quarter's gathered table
    q_a = (src_a % cfg.nloc) // qs
    row_a = (src_a // cfg.nloc) * qs + (src_a % qs)

    order = np.lexsort((dst_a, q_a))
    src_q = q_a[order]
    dst_s = dst_a[order]
    row_s = row_a[order]
    nrm_s = nrm_a[order]
    qstart = np.searchsorted(src_q, np.arange(nq + 1))

    nblk_g = cfg.npad // P
    res = {"T": [], "ttot": []}
    for q in range(nq):
        lo_, hi_ = qstart[q], qstart[q + 1]
        s_r = row_s[lo_:hi_]
        s_d = dst_s[lo_:hi_]
        s_n = nrm_s[lo_:hi_]
        blk = s_d // P
        counts = np.bincount(blk, minlength=nblk_g).reshape(nc_, cfg.nblk)
        T = _cdiv_arr(counts.max(axis=0), P).astype(np.int64)
        ttot = max(int(T.sum()), 1)
        tstart = np.concatenate([[0], np.cumsum(T)]).astype(np.int64)
        bstart = np.concatenate(
            [[0], np.cumsum(np.bincount(blk, minlength=nblk_g))]).astype(np.int64)
        gidx = np.zeros((nc_, P, ttot), np.int16)
        dstc = np.full((nc_, P, ttot), -1.0, np.float32)
        nrmt = np.zeros((nc_, P, ttot), np.float32)
        for c in range(nc_):
            for b in range(cfg.nblk):
                gb = c * cfg.nblk + b
                e0, e1 = bstart[gb], bstart[gb + 1]
                m = e1 - e0
                if m == 0:
                    continue
                jj = np.arange(m)
                pp, tt = jj % P, tstart[b] + jj // P
                gidx[c, pp, tt] = s_r[e0:e1]
                dstc[c, pp, tt] = (s_d[e0:e1] % P).astype(np.float32)
                nrmt[c, pp, tt] = s_n[e0:e1]
        widx = np.stack([_wrap_idx(gidx[c].T.reshape(-1)) for c in range(nc_)])
        res[f"gidx{q}"] = widx
        res[f"dstc{q}"] = dstc.astype(bfd)
        res[f"nrm{q}"] = nrmt.astype(bfd)
        res["T"].append(T)
        res["ttot"].append(ttot)

    # iota repeated per chunk tile: iota[p, t*P + c] = c
    iota = np.tile(np.arange(P, dtype=np.float32), cfg.chunk_tiles)
    res["iota"] = np.ascontiguousarray(
        np.broadcast_to(iota, (P, cfg.chunk_tiles * P))).astype(bfd)

    # per-block diagonal self-loop matrices [P, nblk*P] bf16
    dg = np.zeros((nc_, P, cfg.nblk * P), bfd)
    d2pad = np.zeros(cfg.npad, np.float32)
    d2pad[:n] = dinv2
    for c in range(nc_):
        for b in range(cfg.nblk):
            base = c * cfg.nloc + b * P
            dg[c, np.arange(P), b * P + np.arange(P)] = d2pad[base:base + P].astype(bfd)

    # x transposed per core, padded, bf16
    xT = np.zeros((nc_, cfg.d_in, cfg.nloc), bfd)
    xf = np.asarray(x, dtype=np.float32)
    for c in range(nc_):
        lo2, hi2 = c * cfg.nloc, min((c + 1) * cfg.nloc, n)
        if hi2 > lo2:
            xT[c, :, : hi2 - lo2] = xf[lo2:hi2].T.astype(bfd)

    # pooling matrix with 1/count folded in, block-major [P, nblk*G], bf16
    g_ = cfg.n_graphs
    cnt = np.maximum(np.bincount(batch, minlength=g_).astype(np.float32), 1.0)
    pm = np.zeros((nc_, P, cfg.nblk * g_), bfd)
    for c in range(nc_):
        for b in range(cfg.nblk):
            base = c * cfg.nloc + b * P
            hi2 = min(base + P, n)
            if hi2 <= base:
                continue
            rows = np.arange(hi2 - base)
            gg = batch[base:hi2]
            pm[c, rows, b * g_ + gg] = (1.0 / cnt[gg]).astype(bfd)

    res.update(xT=xT, pm=pm, dg=dg)
    return res


def build(cfg, Ts, ttots):
    """Build the SPMD Bass graph (same program for all cores)."""
    import concourse.mybir as mybir
    import concourse.tile as tile
    from concourse import bacc

    f32 = mybir.dt.float32
    bf = mybir.dt.bfloat16
    i16 = mybir.dt.int16
    AF = mybir.ActivationFunctionType
    ALU = mybir.AluOpType

    nc_ = cfg.n_cores
    nblk = cfg.nblk
    nloc = cfg.nloc
    nq, qs = cfg.nq, cfg.nloc // cfg.nq
    qblk = nblk // nq
    dh = cfg.d_hid
    do = cfg.d_out
    g_ = cfg.n_graphs
    rg = [list(range(nc_))]
    chunk = cfg.chunk_tiles

    nc = bacc.Bacc("TRN2", target_bir_lowering=False, debug=False,
                   num_devices=nc_, num_swdge_queues=4)

    # ---- parameters ----
    xT_p = nc.declare_dram_parameter("xT", [cfg.d_in, nloc], bf, isOutput=False)
    gidx_p, dstc_p, nrm_p = [], [], []
    for q in range(nq):
        gidx_p.append(nc.declare_dram_parameter(
            f"gidx{q}", [P, ttots[q] * 8], i16, isOutput=False))
        dstc_p.append(nc.declare_dram_parameter(
            f"dstc{q}", [P, ttots[q]], bf, isOutput=False))
        nrm_p.append(nc.declare_dram_parameter(
            f"nrm{q}", [P, ttots[q]], bf, isOutput=False))
    iota_p = nc.declare_dram_parameter("iota", [P, chunk * P], bf, isOutput=False)
    dg_p = nc.declare_dram_parameter("dg", [P, nblk * P], bf, isOutput=False)
    pm_p = nc.declare_dram_parameter("pm", [P, nblk * g_], bf, isOutput=False)
    w1_p = nc.declare_dram_parameter("W1", [cfg.d_in, dh], bf, isOutput=False)
    w2_p = nc.declare_dram_parameter("W2", [dh, dh], bf, isOutput=False)
    wl_p = nc.declare_dram_parameter("Wl", [dh, do], f32, isOutput=False)
    b1_p = nc.declare_dram_parameter("b1", [1, dh], bf, isOutput=False)
    b2_p = nc.declare_dram_parameter("b2", [1, dh], bf, isOutput=False)
    bl_p = nc.declare_dram_parameter("bl", [1, do], f32, isOutput=False)
    out_p = nc.declare_dram_parameter("out", [g_, do], f32, isOutput=True)

    # ---- internal DRAM ----
    agin = [nc.dram_tensor(f"agin{l}", [nloc, dh], bf) for l in (1, 2)]
    tables = [[nc.dram_tensor(f"table{l}_{q}", [nc_ * qs, dh], bf,
                              addr_space="Shared") for q in range(nq)]
              for l in (1, 2)]
    arin = nc.dram_tensor("arin", [dh, g_], f32)
    arout = nc.dram_tensor("arout", [dh, g_], f32, addr_space="Shared")

    tstarts = [np.concatenate([[0], np.cumsum(T)]).astype(np.int64) for T in Ts]

    with tile.TileContext(nc) as tc:
        with (
            tc.tile_pool(name="const", bufs=1) as cpool,
            tc.tile_pool(name="big", bufs=1) as bigpool,
            tc.tile_pool(name="gat", bufs=12) as gpool,
            tc.tile_pool(name="m", bufs=6) as mpool,
            tc.tile_pool(name="small", bufs=2) as spool,
            tc.tile_pool(name="psumt", bufs=2, space="PSUM") as pspoolt,
            tc.tile_pool(name="psum", bufs=4, space="PSUM") as pspool,
            tc.tile_pool(name="psum1", bufs=1, space="PSUM") as pspool1,
        ):
            # ---- constants ----
            w1_sb = cpool.tile([cfg.d_in, dh], bf)
            w2_sb = cpool.tile([dh, dh], bf)
            wl_sb = cpool.tile([dh, do], f32)
            b1_sb = cpool.tile([1, dh], bf)
            b2_sb = cpool.tile([1, dh], bf)
            bl_sb = cpool.tile([1, do], f32)
            ones_sb = cpool.tile([1, P], bf)
            onesf_sb = cpool.tile([1, P], f32)
            iota_sb = cpool.tile([P, chunk, P], bf)
            nc.sync.dma_start(w1_sb[:], w1_p[:])
            nc.sync.dma_start(w2_sb[:], w2_p[:])
            nc.sync.dma_start(wl_sb[:], wl_p[:])
            nc.sync.dma_start(b1_sb[:], b1_p[:])
            nc.sync.dma_start(b2_sb[:], b2_p[:])
            nc.sync.dma_start(bl_sb[:], bl_p[:])
            nc.sync.dma_start(
                iota_sb[:].rearrange("p k c -> p (k c)"), iota_p[:])
            nc.gpsimd.memset(ones_sb[:], 1.0)
            nc.gpsimd.memset(onesf_sb[:], 1.0)

            xT_sb = bigpool.tile([cfg.d_in, nloc], bf, tag="lhsT1")
            gidx_sb, dstc_sb, nrm_sb = [], [], []
            for q in range(nq):
                t = bigpool.tile([P, ttots[q] * 8], i16, tag=f"gidx{q}")
                nc.sync.dma_start(t[:], gidx_p[q][:])
                gidx_sb.append(t)
                t = bigpool.tile([P, ttots[q]], bf, tag=f"dstc{q}")
                nc.sync.dma_start(t[:], dstc_p[q][:])
                dstc_sb.append(t)
                t = bigpool.tile([P, ttots[q]], bf, tag=f"nrm{q}")
                nc.sync.dma_start(t[:], nrm_p[q][:])
                nrm_sb.append(t)
            dg_sb = bigpool.tile([P, nblk * P], bf)
            pm_sb = bigpool.tile([P, nblk * g_], bf)
            nc.sync.dma_start(xT_sb[:], xT_p[:])
            nc.sync.dma_start(dg_sb[:], dg_p[:])
            nc.sync.dma_start(pm_sb[:], pm_p[:])

            hpre_sb = bigpool.tile([P, nloc], bf)
            acc_sb = bigpool.tile([P, nloc], f32, tag="acc")
            h1T_sb = bigpool.tile([P, nloc], bf, tag="lhsT2")
            h2_sb = bigpool.tile([P, nloc], bf, tag="hout")

            def bsl(b, w=P):
                return slice(b * w, (b + 1) * w)

            def transform_quarter(lhsT_sb, w_sb, layer, q):
                ag = agin[layer]
                for b in range(q * qblk, (q + 1) * qblk):
                    ps = pspoolt.tile([P, dh], f32, tag="pst")
                    nc.tensor.matmul(ps[:], lhsT_sb[:, bsl(b)], w_sb[:],
                                     start=True, stop=True)
                    nc.vector.tensor_copy(hpre_sb[:, bsl(b)], ps[:])
                nc.sync.dma_start(
                    ag[q * qs:(q + 1) * qs, :].rearrange(
                        "(b p) f -> p b f", p=P),
                    hpre_sb[:, q * qblk * dh:(q + 1) * qblk * dh].rearrange(
                        "p (b f) -> p b f", f=dh))
                nc.gpsimd.collective_compute(
                    "AllGather", mybir.AluOpType.bypass, replica_groups=rg,
                    ins=[ag[q * qs:(q + 1) * qs, :]],
                    outs=[tables[layer][q][:]])

            gq = {"n": 0}

            def aggregate(layer, b_sb, hout_sb, transposed, post_block_cb=None):
                streams = []
                for q in range(nq):
                    streams.append(dict(
                        tstart=tstarts[q], ttot=ttots[q], gidx=gidx_sb[q],
                        dstc=dstc_sb[q], nrm=nrm_sb[q],
                        view=tables[layer][q][:], gcur=None, gc0=-1, mcur=None))

                def fetch(st, t):
                    c0 = (t // chunk) * chunk
                    if st["gc0"] != c0:
                        k = min(chunk, st["ttot"] - c0)
                        gt = gpool.tile([P, chunk, dh], bf, tag="g")
                        nc.gpsimd.dma_gather(
                            out_ap=gt[:, :k, :],
                            in_ap=st["view"],
                            idxs_ap=st["gidx"][:, c0 * 8:(c0 + k) * 8],
                            num_idxs=k * P,
                            num_idxs_reg=k * P,
                            elem_size=dh,
                            queue_num=gq["n"] % 4,
                        )
                        gq["n"] += 1
                        mt = mpool.tile([P, chunk, P], bf, tag="m")
                        dc = st["dstc"][:, c0:c0 + k].unsqueeze(2) \
                            .broadcast_to([P, k, P])
                        nr = st["nrm"][:, c0:c0 + k].unsqueeze(2) \
                            .broadcast_to([P, k, P])
                        nc.vector.tensor_tensor(mt[:, :k, :], iota_sb[:, :k, :],
                                                dc, op=ALU.is_equal)
                        nc.vector.tensor_tensor(mt[:, :k, :], mt[:, :k, :],
                                                nr, op=ALU.mult)
                        st["gcur"], st["mcur"], st["gc0"] = gt, mt, c0
                    j = t - st["gc0"]
                    return st["mcur"][:, j, :], st["gcur"][:, j, :]

                for q in range(nq):
                    st = streams[q]
                    ts = st["tstart"]
                    last = (q == nq - 1)
                    for b in range(nblk):
                        t0, t1 = int(ts[b]), int(ts[b + 1])
                        if t0 == t1 and q > 0:
                            if last:
                                nc.scalar.activation(hout_sb[:, bsl(b)],
                                                     acc_sb[:, bsl(b)], AF.Relu)
                                if post_block_cb:
                                    post_block_cb(b)
                            continue
                        ps = pspool.tile([P, dh], f32, tag="psa")
                        ops = []
                        if q == 0:
                            if transposed:
                                ops.append((hpre_sb[:, bsl(b)], dg_sb[:, bsl(b)]))
                                ops.append((b_sb[:], ones_sb[:]))
                            else:
                                ops.append((dg_sb[:, bsl(b)], hpre_sb[:, bsl(b)]))
                                ops.append((ones_sb[:], b_sb[:]))
                        for t in range(t0, t1):
                            m_ap, g_ap = fetch(st, t)
                            ops.append((g_ap, m_ap) if transposed
                                       else (m_ap, g_ap))
                        for i, (l_ap, r_ap) in enumerate(ops):
                            nc.tensor.matmul(ps[:], l_ap, r_ap,
                                             start=(i == 0),
                                             stop=(i == len(ops) - 1))
                        if q == 0:
                            nc.vector.tensor_copy(acc_sb[:, bsl(b)], ps[:])
                        else:
                            nc.vector.tensor_add(acc_sb[:, bsl(b)],
                                                 acc_sb[:, bsl(b)], ps[:])
                        if last:
                            nc.scalar.activation(hout_sb[:, bsl(b)],
                                                 acc_sb[:, bsl(b)], AF.Relu)
                            if post_block_cb:
                                post_block_cb(b)

            # ---- layer 1: transform + quarter AllGathers ----
            for q in range(nq):
                transform_quarter(xT_sb, w1_sb, 0, q)

            # layer-2 transform is emitted as soon as each quarter of h1T is
            # complete, overlapping its AllGather with remaining aggregation
            def l1_cb(b):
                if (b + 1) % qblk == 0:
                    transform_quarter(h1T_sb, w2_sb, 1, b // qblk)

            aggregate(0, b1_sb, h1T_sb, transposed=True, post_block_cb=l1_cb)

            # ---- layer 2: aggregation (normal orientation) + fused pooling --
            psp = pspool1.tile([P, g_], f32, tag="pool")

            def l2_cb(b):
                nc.tensor.matmul(psp[:], h2_sb[:, bsl(b)], pm_sb[:, bsl(b, g_)],
                                 start=(b == 0), stop=(b == nblk - 1))

            aggregate(1, b2_sb, h2_sb, transposed=False, post_block_cb=l2_cb)

            # ---- pooled sums AllReduce ----
            pool_sb = spool.tile([dh, g_], f32)
            nc.vector.tensor_copy(pool_sb[:], psp[:])
            nc.gpsimd.dma_start(arin[:], pool_sb[:])
            nc.gpsimd.collective_compute(
                "AllReduce", mybir.AluOpType.add, replica_groups=rg,
                ins=[arin[:]], outs=[arout[:]])
            mean_sb = spool.tile([dh, g_], f32)
            nc.sync.dma_start(mean_sb[:], arout[:])

            # ---- final linear + bias ----
            psg = pspool1.tile([g_, do], f32, tag="fin")
            nc.tensor.matmul(psg[:], mean_sb[:], wl_sb[:], start=True, stop=False)
            nc.tensor.matmul(psg[:], onesf_sb[:, :g_], bl_sb[:],
                             start=False, stop=True)
            g_sb = spool.tile([g_, do], f32)
            nc.vector.tensor_copy(g_sb[:], psg[:])

            # ---- L2 normalize rows ----
            sq_sb = spool.tile([g_, do], f32)
            s_sb = spool.tile([g_, 1], f32)
            nrm2_sb = spool.tile([g_, 1], f32)
            inv_sb = spool.tile([g_, 1], f32)
            o_sb = spool.tile([g_, do], f32)
            nc.vector.tensor_mul(sq_sb[:], g_sb[:], g_sb[:])
            nc.vector.tensor_reduce(s_sb[:], sq_sb[:],
                                    axis=mybir.AxisListType.X, op=ALU.add)
            nc.scalar.sqrt(nrm2_sb[:], s_sb[:])
            nc.vector.tensor_scalar_max(nrm2_sb[:], nrm2_sb[:], 1e-12)
            nc.vector.reciprocal(inv_sb[:], nrm2_sb[:])
            nc.vector.tensor_scalar_mul(o_sb[:], g_sb[:], inv_sb[:, :1])
            nc.sync.dma_start(out_p[:], o_sb[:])

    nc.compile()
    return nc


_CACHE = {}
_LAST_EXEC_NS = None


def _run(cfg, x, W1, b1, W2, b2, Wl, bl, edge_index, batch, trace=False):
    import ml_dtypes
    from concourse.bass_utils import run_bass_kernel_spmd
    bfd = ml_dtypes.bfloat16

    pre = preprocess(cfg, x, edge_index, batch)
    key = (cfg.n_nodes, cfg.nloc, tuple(pre["ttot"]),
           tuple(tuple(T.tolist()) for T in pre["T"]))
    if key not in _CACHE:
        _CACHE[key] = build(cfg, pre["T"], pre["ttot"])
    nc = _CACHE[key]

    in_maps = []
    for c in range(cfg.n_cores):
        m = {}
        for q in range(cfg.nq):
            m[f"gidx{q}"] = np.ascontiguousarray(pre[f"gidx{q}"][c])
            m[f"dstc{q}"] = np.ascontiguousarray(pre[f"dstc{q}"][c])
            m[f"nrm{q}"] = np.ascontiguousarray(pre[f"nrm{q}"][c])
        m.update({
            "xT": np.ascontiguousarray(pre["xT"][c]),
            "pm": np.ascontiguousarray(pre["pm"][c]),
            "dg": np.ascontiguousarray(pre["dg"][c]),
            "iota": pre["iota"],
            "W1": np.asarray(W1, np.float32).astype(bfd),
            "W2": np.asarray(W2, np.float32).astype(bfd),
            "Wl": np.asarray(Wl, np.float32),
            "b1": np.asarray(b1, np.float32).astype(bfd).reshape(1, -1),
            "b2": np.asarray(b2, np.float32).astype(bfd).reshape(1, -1),
            "bl": np.asarray(bl, np.float32).reshape(1, -1),
        })
        in_maps.append(m)
    res = run_bass_kernel_spmd(nc, in_maps, list(range(cfg.n_cores)),
                               trace=trace)
    global _LAST_EXEC_NS
    _LAST_EXEC_NS = res.exec_time_ns
    return np.asarray(res.results[0]["out"], np.float32)


def kernel(x, W1, b1, W2, b2, Wl, bl, edge_index, batch):
    cfg = GCNConfig()
    return _run(cfg, x, W1, b1, W2, b2, Wl, bl, edge_index, batch)
